# revision 4
# baseline (speedup 1.0000x reference)
"""Multi-head QKV attention Trainium2 kernel v3 (8-core SPMD).

Problem: B=2, N=M=2048, d_model=256, H=8 heads, d_head=32.
Sharding: core c handles batch b=c//4, heads (2*(c%4), 2*(c%4)+1).
Each core emits a partial [2048, 256] output (its 2 heads through Wo,
summed on-chip); host sums 4 core partials per batch and adds bo.

Structure (all bf16 compute, f32 PSUM accumulate):
- Host pre-transposes x to c-major and pre-casts to bf16; no on-chip
  transposes of inputs, halved input DMA.
- q_rep/k_rep [32g+d, n] built with host-replicated weights (full-K
  matmuls, bias added during the PSUM drain).
- Main loop: 4 passes (n-half x head), 12 m-chunk iterations each.
  Scores PSUM double-buffered; ONE exp instruction per iteration
  ([128,1024] PSUM->SBUF) -- ACT is the designed bottleneck (~55us).
  o' (attn @ v plus a presence column = softmax denominator) accumulates
  into rows 0:33 of a per-pass PSUM tile.
- Everything else rides the per-iteration PE/DVE slack: deferred
  projections (later passes' q_rep/k_rep waves), v projection, and the
  previous pass's epilogue (drain, d-transpose, reciprocal, u = o^T @ Wo,
  normalize + head-accumulate into out_sb, output DMA) are emitted one
  small unit per iteration.
"""

import numpy as np
import ml_dtypes

B, N, C, D = 2, 2048, 256, 32
NCORES = 8
SCALE = 1.0 / np.sqrt(32.0)
BF16 = ml_dtypes.bfloat16

_CACHE = {}


def _build(main_reps=1, nch=12, dbg=False, cut=0):
    """cut: 0=full, 1=prologue only, 2=+deferred units, 3=+passes (no
    epilogues), 4=+tail epilogue only (no deferred epilogues)."""
    key = ("v3k", main_reps, nch, dbg, cut)
    if key in _CACHE:
        return _CACHE[key]

    import concourse.bass as bass  # noqa: F401
    import concourse.bacc as bacc
    import concourse.tile as tile
    from concourse import mybir

    f32 = mybir.dt.float32
    bf16 = mybir.dt.bfloat16
    AF = mybir.ActivationFunctionType
    OP = mybir.AluOpType

    M_pad = nch * 128
    KA = min(1024, M_pad)
    KB = M_pad - KA
    NCHK = 8  # n-chunks of 128 per half
    WBF_COLS = 256 + 256 + 128 + 512 + nch  # wq4 wk4 wv2 wo2 pcolT

    nc = bacc.Bacc(
        "TRN2",
        target_bir_lowering=False,
        debug=False,
        enable_asserts=False,
        num_devices=NCORES,
    )

    wbf_d = nc.dram_tensor("wbf", [128, WBF_COLS], bf16, kind="ExternalInput").ap()
    wf32_d = nc.dram_tensor("wf32", [128, 6], f32, kind="ExternalInput").ap()
    rowp_d = nc.dram_tensor("rowp", [1, 64 + M_pad], bf16, kind="ExternalInput").ap()
    kT_d = nc.dram_tensor("kT", [128, 2, M_pad], bf16, kind="ExternalInput").ap()
    qT_d = nc.dram_tensor("qT", [128, 2, N], bf16, kind="ExternalInput").ap()
    vT_d = nc.dram_tensor("vT", [128, 2, M_pad], bf16, kind="ExternalInput").ap()
    o_d = nc.dram_tensor("od", [128, 16, C], f32, kind="ExternalOutput").ap()

    with tile.TileContext(nc) as tc:
        with (
            tc.tile_pool(name="sb", bufs=1) as sb,
            tc.tile_pool(name="expp", bufs=4) as expp,
            tc.tile_pool(name="ep", bufs=2) as ep,
            tc.tile_pool(name="ps", bufs=1, space="PSUM") as ps,
        ):
            # ---- DMAs (order = need order) ----
            wbf = sb.tile([128, WBF_COLS], bf16, tag="wbf")
            nc.sync.dma_start(out=wbf[:, 0:512], in_=wbf_d[:, 0:512])
            wf32 = sb.tile([128, 6], f32, tag="wf32")
            nc.gpsimd.dma_start(out=wf32, in_=wf32_d)
            rowp = sb.tile([1, 64 + M_pad], bf16, tag="rowp")
            nc.gpsimd.dma_start(out=rowp, in_=rowp_d)
            kT = sb.tile([128, 2, M_pad], bf16, tag="kT")
            nc.sync.dma_start(out=kT[:, :, 0:512], in_=kT_d[:, :, 0:512])
            qT = sb.tile([128, 2, N], bf16, tag="qT")
            nc.sync.dma_start(out=qT[:, :, 0:1024], in_=qT_d[:, :, 0:1024])
            nc.sync.dma_start(out=wbf[:, 512:], in_=wbf_d[:, 512:])
            if KA > 512:
                nc.sync.dma_start(out=kT[:, :, 512:KA], in_=kT_d[:, :, 512:KA])
            vT = sb.tile([128, 2, M_pad], bf16, tag="vT")
            nc.sync.dma_start(out=vT[:, :, 0:KA], in_=vT_d[:, :, 0:KA])
            if KB:
                nc.sync.dma_start(out=vT[:, :, KA:M_pad], in_=vT_d[:, :, KA:M_pad])
                nc.sync.dma_start(out=kT[:, :, KA:M_pad], in_=kT_d[:, :, KA:M_pad])
            nc.sync.dma_start(out=qT[:, :, 1024:2048], in_=qT_d[:, :, 1024:2048])

            wq4 = wbf[:, 0:256].rearrange("p (ch h g) -> p ch h g", ch=2, h=2)
            wk4 = wbf[:, 256:512].rearrange("p (ch h g) -> p ch h g", ch=2, h=2)
            wv2 = wbf[:, 512:640].rearrange("p (ch e) -> p ch e", ch=2)
            wo2 = wbf[:, 640:1152]
            pcolT = wbf[:, 1152 : 1152 + nch]
            bq4 = wf32[:, 0:2]
            bk4 = wf32[:, 2:4]
            ident1 = wf32[0:1, 4:5]
            bv2 = rowp[0:1, 0:64]
            prow = rowp[0:1, 64 : 64 + M_pad]

            # preload ACT exp table while DMAs run
            scr = sb.tile([128, 4], bf16, tag="scr")
            nc.scalar.activation(out=scr, in_=wf32[:, 0:4], func=AF.Exp, bias=0.0, scale=1.0)

            # ---- persistent SBUF ----
            k_rep = [sb.tile([64, M_pad], bf16, tag=f"kr{h}", name=f"kr{h}") for h in range(2)]
            q_rep = [
                [sb.tile([64, 1024], bf16, tag=f"qr{h}{hf}", name=f"qr{h}{hf}") for hf in range(2)]
                for h in range(2)
            ]
            v_aug = sb.tile([128, nch, 2, D + 1], bf16, tag="vaug")

            # ---- projection helpers ----
            def proj_unit(dst, wrep, brep, h, src, src_off, cols, eng, uname):
                """One [128,<=512] projection: 2 full-K MMs + bias drain."""
                pp = ps.tile([128, 512], f32, tag="U", bufs=2, name=f"pp_{uname}")
                for ch in range(2):
                    nc.tensor.matmul(
                        pp[0:64, 0:cols],
                        lhsT=wrep[:, ch, h, :],
                        rhs=src[:, ch, src_off : src_off + cols],
                        start=(ch == 0),
                        stop=(ch == 1),
                    )
                if eng is nc.vector:
                    nc.vector.tensor_scalar(
                        out=dst, in0=pp[0:64, 0:cols], scalar1=brep[0:64, h : h + 1],
                        scalar2=None, op0=OP.add,
                    )
                else:
                    nc.scalar.activation(
                        out=dst, in_=pp[0:64, 0:cols], func=AF.Identity,
                        bias=brep[0:64, h : h + 1], scale=1.0,
                    )

            def v_unit(m0, m1, uname):
                vp = ps.tile([128, 512], f32, tag="U", bufs=2, name=f"vp_{uname}")
                for mc in range(m0, m1):
                    off = 64 * (mc - m0)
                    for ch in range(2):
                        nc.tensor.matmul(
                            vp[:, off : off + 64],
                            lhsT=vT[:, ch, 128 * mc : 128 * mc + 128],
                            rhs=wv2[:, ch, :],
                            start=(ch == 0),
                            stop=False,
                        )
                    nc.tensor.matmul(
                        vp[:, off : off + 64],
                        lhsT=prow[0:1, 128 * mc : 128 * mc + 128],
                        rhs=bv2,
                        start=False,
                        stop=True,
                    )
                nchu = m1 - m0
                nc.vector.tensor_copy(
                    out=v_aug[:, m0:m1, :, 0:D],
                    in_=vp[:, 0 : 64 * nchu].rearrange("p (mc h e) -> p mc h e", h=2, e=D),
                )

            # ---- prologue: only what pass 0 iteration 0 needs ----
            # k_rep[0] first 512 (covers mc 0-3), q_rep[0][0] full half, then
            # k_rep[0] cols 512:KA. Finer splits let the first exp start as
            # soon as the first kT/qT DMA slices land.
            pro_k0 = ps.tile([128, 512], f32, tag="sc", bufs=2, name="prok0a")
            for ch in range(2):
                nc.tensor.matmul(
                    pro_k0[0:64, 0:512], lhsT=wk4[:, ch, 0, :], rhs=kT[:, ch, 0:512],
                    start=(ch == 0), stop=(ch == 1),
                )
            nc.vector.tensor_scalar(
                out=k_rep[0][:, 0:512], in0=pro_k0[0:64, 0:512],
                scalar1=bk4[0:64, 0:1], scalar2=None, op0=OP.add,
            )
            pro_q = ps.tile([128, 1024], f32, tag="sc", bufs=2, name="proq00")
            for s in range(0, 1024, 512):
                for ch in range(2):
                    nc.tensor.matmul(
                        pro_q[0:64, s : s + 512],
                        lhsT=wq4[:, ch, 0, :],
                        rhs=qT[:, ch, s : s + 512],
                        start=(ch == 0),
                        stop=(ch == 1),
                    )
            nc.scalar.activation(
                out=q_rep[0][0], in_=pro_q[0:64, :], func=AF.Identity,
                bias=bq4[0:64, 0:1], scale=1.0,
            )
            if KA > 512:
                pro_k1 = ps.tile([128, 512], f32, tag="sc", bufs=2, name="prok0b")
                for ch in range(2):
                    nc.tensor.matmul(
                        pro_k1[0:64, 0 : KA - 512], lhsT=wk4[:, ch, 0, :],
                        rhs=kT[:, ch, 512:KA], start=(ch == 0), stop=(ch == 1),
                    )
                nc.vector.tensor_scalar(
                    out=k_rep[0][:, 512:KA], in0=pro_k1[0:64, 0 : KA - 512],
                    scalar1=bk4[0:64, 0:1], scalar2=None, op0=OP.add,
                )

            # ---- deferred work units ----
            from collections import deque

            pending = deque()

            def v0_and_pcol():
                v_unit(0, min(8, nch), "v0")
                for h in range(2):
                    nc.vector.tensor_copy(
                        out=v_aug[:, :, h, D : D + 1].rearrange("p a b -> p (a b)"),
                        in_=pcolT,
                    )

            pending.append(v0_and_pcol)
            if nch > 8:
                pending.append(lambda: v_unit(8, nch, "v1"))
            # k_rep head0 tail (needed at iter 8 of pass 0!), then q_rep[1][0]
            # and k_rep[1] (needed at pass 1), then half-1 q_reps (passes 2/3)
            if KB:
                for s in range(KA, M_pad, 512):
                    pending.append(
                        lambda s=s: proj_unit(
                            k_rep[0][:, s : s + min(512, M_pad - s)], wk4, bk4, 0,
                            kT, s, min(512, M_pad - s), nc.vector, f"k0_{s}",
                        )
                    )
            for s in range(0, 1024, 512):
                pending.append(
                    lambda s=s: proj_unit(
                        q_rep[1][0][:, s : s + 512], wq4, bq4, 1,
                        qT, s, 512, nc.vector, f"q10_{s}",
                    )
                )
            for s in range(0, M_pad, 512):
                pending.append(
                    lambda s=s: proj_unit(
                        k_rep[1][:, s : s + min(512, M_pad - s)], wk4, bk4, 1,
                        kT, s, min(512, M_pad - s), nc.vector, f"k1_{s}",
                    )
                )
            for h in range(2):
                for s in range(0, 1024, 512):
                    pending.append(
                        lambda h=h, s=s: proj_unit(
                            q_rep[h][1][:, s : s + 512], wq4, bq4, h,
                            qT, 1024 + s, 512, nc.vector, f"q{h}1_{s}",
                        )
                    )

            # ---- passes ----
            out_sb = [
                ep.tile([128, NCHK, C], f32, tag="outsb", name=f"outsb{hf}")
                for hf in range(2)
            ]
            dbg_tiles = {}

            def epilogue_units(pi, hf, h, oacc, tail):
                """Emit-or-defer the epilogue of pass (hf, h).

                Non-tail: normalize on DVE, second head accumulates into
                out_sb (stt), plain DMA out. Tail: normalize split ACT/DVE
                into its own buffer, DMA with accum_op=add on top of the
                already-written first-head partial (Tile orders the DMAs).
                """
                o_sb = ep.tile([128, 1024], bf16, tag="osb", name=f"osb{pi}")
                d2 = ep.tile([1, 1024], f32, tag="d2", name=f"d2{pi}")
                r_sb = ep.tile([128, 8], f32, tag="rsb", name=f"rsb{pi}")
                dbg_tiles[pi] = (o_sb, d2, r_sb)
                dst = out_sb[hf]
                units = []

                def drains():
                    # d2 first: it heads the critical chain (transpose->recip)
                    nc.vector.tensor_copy(out=d2, in_=oacc[32:33, :])
                    if tail:
                        nc.scalar.copy(out=o_sb[0:33, :], in_=oacc[0:33, :])
                    else:
                        nc.vector.tensor_copy(out=o_sb[0:33, :], in_=oacc[0:33, :])

                units.append(drains)

                def u_chunk(j):
                    u = ps.tile([128, 512], f32, tag="U", bufs=2, name=f"u{pi}_{j}")
                    if j == 0:
                        for jj in range(NCHK):
                            nc.tensor.transpose(
                                out=u[:, 256 + jj : 256 + jj + 1],
                                in_=d2[0:1, 128 * jj : 128 * jj + 128],
                                identity=ident1,
                            )
                        nc.vector.reciprocal(out=r_sb, in_=u[:, 256 : 256 + NCHK])
                    nc.tensor.matmul(
                        u[:, 0:256],
                        lhsT=o_sb[0:32, 128 * j : 128 * j + 128],
                        rhs=wo2[0:32, 256 * h : 256 * h + 256],
                        start=True,
                        stop=True,
                    )
                    if h == 0:
                        nc.vector.tensor_scalar(
                            out=dst[:, j, :], in0=u[:, 0:256],
                            scalar1=r_sb[:, j : j + 1], scalar2=None, op0=OP.mult,
                        )
                    else:
                        nc.vector.scalar_tensor_tensor(
                            out=dst[:, j, :], in0=u[:, 0:256],
                            scalar=r_sb[:, j : j + 1],
                            in1=dst[:, j, :], op0=OP.mult, op1=OP.add,
                        )
                    if h == 1 and j in (3, 5, NCHK - 1):
                        j0 = {3: 0, 5: 4, NCHK - 1: 6}[j]
                        nc.sync.dma_start(
                            out=o_d[:, 8 * hf + j0 : 8 * hf + j + 1, :],
                            in_=dst[:, j0 : j + 1, :],
                        )

                for j in range(NCHK):
                    units.append(lambda j=j: u_chunk(j))
                return units

            fifo = deque()
            NR = 32 if cut == 6 else 33

            def emit_oprime(oacc, h, ex, mc, first, last):
                if cut == 5:
                    return
                for j in range(2):
                    nc.tensor.matmul(
                        oacc[0:NR, 512 * j : 512 * j + 512],
                        lhsT=v_aug[:, mc, h, 0:NR],
                        rhs=ex[:, 512 * j : 512 * j + 512],
                        start=first,
                        stop=last,
                        skip_group_check=True,
                    )

            if cut:
                dump = ep.tile([128, NCHK, C], f32, tag="outsb", name="cutdump")
                nc.vector.memset(dump, 0.0)
                if cut == 2:
                    while pending:
                        pending.popleft()()
                for j in range(16):
                    nc.sync.dma_start(out=o_d[:, j : j + 1, :], in_=dump[:, 0:1, :])
            run_passes = cut in (0, 3, 4, 5, 6, 7, 8)

            pending_epi = None
            for rep in range(main_reps if run_passes else 0):
                last_rep = rep == main_reps - 1
                for pi, (hf, h) in enumerate([(0, 0), (0, 1), (1, 0), (1, 1)]):
                    oacc = ps.tile([128, 1024], f32, tag="O", name=f"oacc{rep}_{pi}")

                    for mc in range(nch):
                        # previous pass's epilogue may only be emitted once
                        # its last o' has been emitted (Tile derives deps
                        # from emission order — a read emitted before its
                        # writer races it)
                        if pending_epi is not None and not any(
                            e[0] is not oacc for e in fifo
                        ):
                            pending.extend(epilogue_units(*pending_epi))
                            pending_epi = None
                        sc = ps.tile(
                            [128, 1024], f32, tag="sc", bufs=2,
                            name=f"sc{rep}_{pi}_{mc}",
                        )
                        if cut == 8:
                            nc.vector.memset(sc, 0.5)
                        else:
                            # 2-way row tiling: each concurrent tile-position
                            # writes one full PSUM bank (concurrent matmuls
                            # into the SAME bank are a hardware PSUM fault)
                            for g in range(2):
                                nc.tensor.matmul(
                                    sc[:, 512 * g : 512 * g + 512],
                                    lhsT=k_rep[h][32 * g : 32 * g + 32, 128 * mc : 128 * mc + 128],
                                    rhs=q_rep[h][hf][32 * g : 32 * g + 32, 512 * g : 512 * g + 512],
                                    start=True,
                                    stop=True,
                                    tile_position=(32 * g, 0),
                                )
                        ex = expp.tile(
                            [128, 1024], bf16, tag="exp", name=f"ex{rep}_{pi}_{mc}",
                        )
                        if cut == 7:
                            nc.vector.tensor_copy(out=ex, in_=sc)
                        else:
                            nc.scalar.activation(
                                out=ex, in_=sc, func=AF.Exp, bias=0.0, scale=float(SCALE),
                            )
                        # lag-2 software pipeline crossing pass boundaries:
                        # o'(i) lands after sc(i+2), so neither the o' flush
                        # nor the next pass's oacc WAR wait stalls PE
                        fifo.append((oacc, h, ex, mc, mc == 0, mc == nch - 1))
                        if len(fifo) > 2:
                            emit_oprime(*fifo.popleft())
                        if pending:
                            pending.popleft()()
                    tail = pi == 3 and last_rep
                    if cut in (3, 5, 6, 7, 8):
                        if tail:
                            while fifo:
                                emit_oprime(*fifo.popleft())
                    elif tail:
                        while fifo:
                            emit_oprime(*fifo.popleft())
                        for u in epilogue_units(rep * 4 + pi, hf, h, oacc, tail):
                            u()
                    elif cut != 4:
                        pending_epi = (rep * 4 + pi, hf, h, oacc, tail)
            while pending:
                pending.popleft()()

            if dbg:
                for h in range(2):
                    for hf in range(2):
                        dqd = nc.dram_tensor(
                            f"dbg_qr{h}{hf}", [128, 1024], bf16, kind="ExternalOutput"
                        ).ap()
                        nc.sync.dma_start(out=dqd, in_=q_rep[h][hf])
                    dkd = nc.dram_tensor(
                        f"dbg_kr{h}", [128, M_pad], bf16, kind="ExternalOutput"
                    ).ap()
                    nc.sync.dma_start(out=dkd, in_=k_rep[h])
                dvd = nc.dram_tensor(
                    "dbg_va", [128, nch * 2 * (D + 1)], bf16, kind="ExternalOutput"
                ).ap()
                nc.sync.dma_start(
                    out=dvd, in_=v_aug.rearrange("p a b c -> p (a b c)")
                )
                for pi in (2, 3):
                    osb, d2t, rsbt = dbg_tiles[pi]
                    dod = nc.dram_tensor(
                        f"dbg_osb{pi}", [33, 1024], bf16, kind="ExternalOutput"
                    ).ap()
                    nc.sync.dma_start(out=dod, in_=osb[0:33, :])
                    ddd = nc.dram_tensor(
                        f"dbg_d2{pi}", [1, 1024], f32, kind="ExternalOutput"
                    ).ap()
                    nc.sync.dma_start(out=ddd, in_=d2t)
                    drd = nc.dram_tensor(
                        f"dbg_r{pi}", [128, 8], f32, kind="ExternalOutput"
                    ).ap()
                    nc.sync.dma_start(out=drd, in_=rsbt)

    nc.compile()
    _CACHE[key] = nc
    return nc


def _plan_compaction(presence):
    idxs = [np.where(np.asarray(presence[b]) > 0)[0] for b in range(B)]
    mc = max(len(ix) for ix in idxs)
    nch = min(16, max(4, 4 * ((mc + 511) // 512)))
    return idxs, nch


def _core_inputs(inputs, c, idxs, nch):
    b, p = c // 4, c % 4
    h0 = 2 * p
    hsl = slice(h0 * D, (h0 + 2) * D)
    f = np.float32
    M_pad = nch * 128

    def cmajor(x):
        # [rows, 256] -> [128, 2, rows]: out[p_, ch, r] = x[r, 128*ch+p_]
        return np.ascontiguousarray(
            x.T.reshape(2, 128, -1).transpose(1, 0, 2)
        ).astype(BF16)

    idx = idxs[b]
    nk = len(idx)
    keys_c = np.zeros((M_pad, C), f)
    keys_c[:nk] = np.asarray(inputs["keys"], f)[b][idx]
    values_c = np.zeros((M_pad, C), f)
    values_c[:nk] = np.asarray(inputs["values"], f)[b][idx]
    pres = np.zeros(M_pad, f)
    pres[:nk] = 1.0

    Wq, Wk, Wv, Wo = (np.asarray(inputs[k], f) for k in ("Wq", "Wk", "Wv", "Wo"))
    bq, bk, bv = (np.asarray(inputs[k], f) for k in ("bq", "bk", "bv"))

    def wrep(W):
        # [128, 2, 2, 64]: out[p_, ch, h, 32g+d] = W[128ch+p_, 32(h0+h)+d]
        ws = W[:, hsl].reshape(2, 128, 2, D)  # [ch, p, h, d]
        return np.tile(ws.transpose(1, 0, 2, 3), (1, 1, 1, 2)).astype(BF16)

    wq4 = wrep(Wq).reshape(128, 256)
    wk4 = wrep(Wk).reshape(128, 256)
    wv2 = (
        np.ascontiguousarray(Wv[:, hsl].reshape(2, 128, 2 * D).transpose(1, 0, 2))
        .astype(BF16)
        .reshape(128, 128)
    )
    # wo2: [128, 512]; rows 0:32 hold Wo for head h at cols 256h:256h+256
    wo2 = np.zeros((128, 512), f)
    wo2[0:32, 0:256] = Wo[hsl][0:D]
    wo2[0:32, 256:512] = Wo[hsl][D : 2 * D]
    wo2 = wo2.astype(BF16)
    pcolT = np.ascontiguousarray(pres.reshape(nch, 128).T).astype(BF16)
    wbf = np.concatenate([wq4, wk4, wv2, wo2, pcolT], axis=1)

    wf32 = np.zeros((128, 6), f)
    for h in range(2):
        wf32[:, h] = np.tile(bq[hsl][h * D : (h + 1) * D], 4)
        wf32[:, 2 + h] = np.tile(bk[hsl][h * D : (h + 1) * D], 4)
    wf32[0, 4] = 1.0  # ident1

    rowp = np.zeros((1, 64 + M_pad), f)
    rowp[0, 0:64] = bv[hsl]
    rowp[0, 64:] = pres

    return {
        "wbf": wbf,
        "wf32": wf32,
        "rowp": rowp.astype(BF16),
        "kT": cmajor(keys_c),
        "qT": cmajor(np.asarray(inputs["queries"], f)[b]),
        "vT": cmajor(values_c),
    }


def make_in_maps(inputs):
    idxs, nch = _plan_compaction(np.asarray(inputs["presence"]))
    return [_core_inputs(inputs, c, idxs, nch) for c in range(NCORES)], nch


def kernel(**inputs):
    from concourse.bass_utils import run_bass_kernel_spmd

    in_maps, nch = make_in_maps(inputs)
    nc = _build(nch=nch)
    res = run_bass_kernel_spmd(nc, in_maps, core_ids=list(range(NCORES)))
    bo = np.asarray(inputs["bo"], np.float32)
    out = np.zeros((B, N, C), np.float32)
    for c in range(NCORES):
        out[c // 4] += res.results[c]["od"].transpose(1, 0, 2).reshape(N, C)
    out += bo[None, None, :]
    return out
